# revision 9
# baseline (speedup 1.0000x reference)
"""BlockSparseLinear on 8 TRN2 NeuronCores.

Computes out = x @ W_dense.T + bias where W_dense is a [4096, 4096] matrix
assembled from 8192 nonzero 32x32 blocks (50% density).

Strategy:
  - Host: scatter the nonzero blocks into a dense weight, scale by 32 (keeps
    the fp8 section of W out of e4m3 subnormals), lay out per-core shards in
    the transposed/tiled device layout, and divide the device output by 32.
  - Sharding: 4-way over tokens x 2-way over out-features (8 cores).
    Per core: out_shard[1024 tokens, 2048 outf] = x_shard @ W_half.T + bias.
  - Mixed precision chosen from measured PE rates (fp32r 227ns, fp16 216ns,
    fp8-DoubleRow 216ns per 512-moving-row matmul, where one DoubleRow
    instruction contracts TWO 128-deep k-planes = 2x fp16 throughput):
    k-tiles 0..23 run in fp16, k-tiles 24..31 run in fp8e4m3 DoubleRow.
    Host-simulated end-to-end rel err vs the fp32 reference: 1.60e-2, inside
    the 2e-2 gate with margin. Trades 128 of 1024 matmul instructions away.
  - Loop structure (from trace analysis):
      Phase A  (kb-major, fp16, k-tiles 0..15): sweep all 16 o-tiles per kb
        of 8 k-tiles, accumulate psum -> SBUF acc via DVE (bias folded in).
      Phase A2 (kb-style, fp8 DoubleRow, k-tiles 24..31): 4 DR matmuls per
        (o-tile, token-chunk), DVE-accumulated into acc.
      Phase B  (o-tile-major, fp16, k-tiles 16..23): one 8-step PSUM-resident
        accumulation per o-tile, final DVE add, then that o-tile's out DMA
        IMMEDIATELY - spreading the 8MB out flush over the whole phase
        instead of backloading it at the HBM write ceiling.
    All x is SBUF-resident before phase A2 (fp16 48KB/part + fp8 8KB/part).
  - Preamble hiding: a dozen warmup matmuls on memset tiles ramp the PE
    p-state (0.65 -> 1.2 -> 2.4GHz takes ~5us of continuous busy) while the
    first real DMAs are in flight; the first W tile is DMA'd in per-k8 32KB
    slices so the first real matmul only waits for the first slice.
"""

import os

import numpy as np
import ml_dtypes

import concourse.mybir as mybir
import concourse.tile as tile
from concourse import bacc
from concourse.bass_utils import run_bass_kernel_spmd

BLOCK = 32
IN_FEATURES = 4096
OUT_FEATURES = 4096
N_TOKENS = 4096
IN_BLOCKS = IN_FEATURES // BLOCK  # 128
OUT_BLOCKS = OUT_FEATURES // BLOCK  # 128

N_CORES = 8
T_SHARDS = 4  # token shards
O_SHARDS = 2  # out-feature shards
TSH = N_TOKENS // T_SHARDS  # 1024 tokens per core
OSH = OUT_FEATURES // O_SHARDS  # 2048 out features per core

P = 128  # partitions
NFREE = 512  # matmul moving free dim (one PSUM bank of fp32)
K_TILES = IN_FEATURES // P  # 32
T_CHUNKS = TSH // NFREE  # 2 moving token chunks per core
O_TILES = OSH // P  # 16 o-tiles of 128 outf
KB_SIZE = 8  # k-tiles per fp16 w tile / phase group

A_GROUPS = 2  # fp16 kb-major phases: k-tiles 0..15
B_K0 = A_GROUPS * KB_SIZE  # phase B fp16 k-tiles 16..23
FP8_K0 = B_K0 + KB_SIZE  # fp8 k-tiles 24..31
FP8_PAIRS = (K_TILES - FP8_K0) // 2  # 4 DoubleRow pairs
K16_TILES = FP8_K0  # 24 fp16 k-tiles

WSCALE = 32.0  # host-side weight scale (undone on the host after gather)
N_WARMUP_MM = 8  # p-state ramp matmuls issued before the first real one

# exec time of the slowest core from the last traced run (ns), None if untraced
LAST_EXEC_NS = None
LAST_RESULT = None


def _install_axon_ntff_hook():
    """Best-effort: register the axon NTFF profiling hook that the image's
    antenv package lacks. Returns True if tracing is possible."""
    try:
        from antenv.axon_hooks import get_axon_ntff_profile_hook

        return get_axon_ntff_profile_hook() is not None
    except ImportError:
        pass
    try:
        import sys
        import types

        import antenv
        import trn_agent_boot.trn_boot as tb

        hook = tb._ntff_profile_via_ctypes("/opt/axon/libaxon_pjrt.so")
        if hook is None:
            return False
        mod = types.ModuleType("antenv.axon_hooks")
        mod._hook = hook
        mod.get_axon_ntff_profile_hook = lambda: mod._hook
        mod.set_axon_ntff_profile_hook = lambda h: setattr(mod, "_hook", h)
        sys.modules["antenv.axon_hooks"] = mod
        antenv.axon_hooks = mod

        # avoid the artifact-upload dependency in the trace path
        import concourse.bass_utils as bu

        bu.upload_artifacts = lambda tmpdir: str(tmpdir)
        return True
    except Exception:
        return False


def _build_bass():
    nc = bacc.Bacc(None, target_bir_lowering=False)

    x_d = nc.dram_tensor(
        "xt", [P, K16_TILES, TSH], mybir.dt.float16, kind="ExternalInput"
    )
    # x8[p, pair, i, t] = x[t0 + t, (FP8_K0 + 2*pair + i)*128 + p]  (fp8)
    x8_d = nc.dram_tensor(
        "x8", [P, FP8_PAIRS, 2, TSH], mybir.dt.float8e4, kind="ExternalInput"
    )
    # wt[g, ot, p(k), k8, o] = Ws[o0 + ot*128 + o, (g*KB_SIZE + k8)*128 + p]
    w_d = nc.dram_tensor(
        "wt",
        [K16_TILES // KB_SIZE, O_TILES, P, KB_SIZE, P],
        mybir.dt.float16,
        kind="ExternalInput",
    )
    # w8[ot, p, pair, i, o] = Ws[o0 + ot*128 + o, (FP8_K0 + 2*pair + i)*128 + p]
    w8_d = nc.dram_tensor(
        "w8", [O_TILES, P, FP8_PAIRS, 2, P], mybir.dt.float8e4, kind="ExternalInput"
    )
    b_d = nc.dram_tensor("bias", [P, O_TILES], mybir.dt.float32, kind="ExternalInput")
    o_d = nc.dram_tensor(
        "out", [O_TILES, P, TSH], mybir.dt.float32, kind="ExternalOutput"
    )

    with tile.TileContext(nc) as tc:
        with (
            tc.tile_pool(name="xpool", bufs=K16_TILES * T_CHUNKS) as xpool,
            tc.tile_pool(name="x8pool", bufs=FP8_PAIRS * T_CHUNKS) as x8pool,
            tc.tile_pool(name="wpool", bufs=12) as wpool,
            tc.tile_pool(name="w8pool", bufs=4) as w8pool,
            tc.tile_pool(name="apool", bufs=1) as apool,
            tc.tile_pool(name="bpool", bufs=1) as bpool,
            tc.tile_pool(name="warm", bufs=1) as wupool,
            tc.tile_pool(name="psum", bufs=7, space="PSUM") as ppool,
            tc.tile_pool(name="psumw", bufs=1, space="PSUM") as pwpool,
        ):
            # PE p-state warmup: matmuls on memset junk, issued before any
            # real matmul; they execute while the first DMAs are in flight.
            wu_w = wupool.tile([P, P], mybir.dt.float16)
            wu_x = wupool.tile([P, NFREE], mybir.dt.float16)
            nc.gpsimd.memset(wu_w[:], 0.0)
            nc.gpsimd.memset(wu_x[:], 0.0)
            wu_ps = pwpool.tile([P, NFREE], mybir.dt.float32, tag="wu", name="wups")
            for _ in range(N_WARMUP_MM):
                nc.tensor.matmul(
                    wu_ps[:], lhsT=wu_w[:], rhs=wu_x[:], start=True, stop=True
                )

            bias_sb = bpool.tile([P, O_TILES], mybir.dt.float32)

            acc_tiles = [
                apool.tile([P, TSH], mybir.dt.float32, tag=f"a{ot}", name="acc")
                for ot in range(O_TILES)
            ]

            # fp16 x chunk tiles (k-tiles 0..23) + fp8 pair tiles, all resident
            x_tiles = [[None] * T_CHUNKS for _ in range(K16_TILES)]
            x8_tiles = [[None] * T_CHUNKS for _ in range(FP8_PAIRS)]

            def load_x(k):
                for tcn in range(T_CHUNKS):
                    x_k = xpool.tile([P, NFREE], mybir.dt.float16, tag="x", name="x")
                    nc.scalar.dma_start(
                        x_k[:], x_d[:, k, tcn * NFREE : (tcn + 1) * NFREE]
                    )
                    x_tiles[k][tcn] = x_k

            def dve_accum(ot, psums, first):
                acc = acc_tiles[ot]
                for tcn in range(T_CHUNKS):
                    sl = slice(tcn * NFREE, (tcn + 1) * NFREE)
                    if first:
                        nc.vector.tensor_tensor(
                            acc[:, sl],
                            psums[tcn][:],
                            bias_sb[:, ot : ot + 1].to_broadcast([P, NFREE]),
                            mybir.AluOpType.add,
                        )
                    else:
                        nc.vector.tensor_tensor(
                            acc[:, sl], psums[tcn][:], acc[:, sl], mybir.AluOpType.add
                        )

            # ---- Phase A: kb-major fp16 over k-tiles 0..15 ----
            for kb in range(A_GROUPS):
                for k8 in range(KB_SIZE):
                    load_x(kb * KB_SIZE + k8)
                if kb == 0:
                    nc.scalar.dma_start(bias_sb[:], b_d[:])
                if kb == A_GROUPS - 1:
                    # prefetch everything later phases need, in consumption
                    # order: fp8 pair tiles (phase A2), then phase B x
                    for pair in range(FP8_PAIRS):
                        for tcn in range(T_CHUNKS):
                            x8_k = x8pool.tile(
                                [P, 2, NFREE], mybir.dt.float8e4, tag="x8", name="x8"
                            )
                            nc.scalar.dma_start(
                                x8_k[:],
                                x8_d[:, pair, :, tcn * NFREE : (tcn + 1) * NFREE],
                            )
                            x8_tiles[pair][tcn] = x8_k
                    for k in range(B_K0, K16_TILES):
                        load_x(k)
                for ot in range(O_TILES):
                    w_sb = wpool.tile(
                        [P, KB_SIZE, P], mybir.dt.float16, tag="w", name="w"
                    )
                    if kb == 0 and ot < 2:
                        # per-k8 slices so early matmuls wait for 32KB each
                        for k8 in range(KB_SIZE):
                            nc.sync.dma_start(w_sb[:, k8], w_d[kb, ot, :, k8])
                    else:
                        nc.sync.dma_start(w_sb[:], w_d[kb, ot])
                    psums = [
                        ppool.tile([P, NFREE], mybir.dt.float32, tag="acc", name="ps")
                        for _ in range(T_CHUNKS)
                    ]
                    for k8 in range(KB_SIZE):
                        for tcn in range(T_CHUNKS):
                            nc.tensor.matmul(
                                psums[tcn][:],
                                lhsT=w_sb[:, k8],
                                rhs=x_tiles[kb * KB_SIZE + k8][tcn][:],
                                start=(k8 == 0),
                                stop=(k8 == KB_SIZE - 1),
                            )
                    dve_accum(ot, psums, first=(kb == 0))

            # ---- Phase A2: fp8 DoubleRow over k-tiles 24..31 ----
            for ot in range(O_TILES):
                w8_sb = w8pool.tile(
                    [P, FP8_PAIRS, 2, P], mybir.dt.float8e4, tag="w8", name="w8"
                )
                nc.sync.dma_start(w8_sb[:], w8_d[ot])
                psums = [
                    ppool.tile([P, NFREE], mybir.dt.float32, tag="acc", name="ps")
                    for _ in range(T_CHUNKS)
                ]
                for pair in range(FP8_PAIRS):
                    for tcn in range(T_CHUNKS):
                        nc.tensor.matmul(
                            psums[tcn][:],
                            lhsT=w8_sb[:, pair],
                            rhs=x8_tiles[pair][tcn][:],
                            start=(pair == 0),
                            stop=(pair == FP8_PAIRS - 1),
                            perf_mode=mybir.MatmulPerfMode.DoubleRow,
                        )
                dve_accum(ot, psums, first=False)

            # ---- Phase B: o-tile-major fp16 over k-tiles 16..23, out DMA per
            # o-tile as soon as it completes ----
            for ot in range(O_TILES):
                w_sb = wpool.tile([P, KB_SIZE, P], mybir.dt.float16, tag="w", name="w")
                nc.sync.dma_start(w_sb[:], w_d[A_GROUPS, ot])
                psums = [
                    ppool.tile([P, NFREE], mybir.dt.float32, tag="acc", name="ps")
                    for _ in range(T_CHUNKS)
                ]
                acc = acc_tiles[ot]
                # tcn-major: chunk 0's DVE add + out DMA overlap chunk 1's MMs
                for tcn in range(T_CHUNKS):
                    for k8 in range(KB_SIZE):
                        nc.tensor.matmul(
                            psums[tcn][:],
                            lhsT=w_sb[:, k8],
                            rhs=x_tiles[B_K0 + k8][tcn][:],
                            start=(k8 == 0),
                            stop=(k8 == KB_SIZE - 1),
                        )
                    sl = slice(tcn * NFREE, (tcn + 1) * NFREE)
                    nc.vector.tensor_tensor(
                        acc[:, sl], psums[tcn][:], acc[:, sl], mybir.AluOpType.add
                    )
                    nc.scalar.dma_start(o_d[ot, :, sl], acc[:, sl])

    nc.compile()
    return nc


def _dense_weight(weight_data, block_ids):
    """Scatter nonzero 32x32 blocks into dense [OUT, IN] (numpy, host-side)."""
    w = np.zeros((OUT_FEATURES, IN_FEATURES), dtype=np.float32)
    br = block_ids.astype(np.int64) // IN_BLOCKS
    bc = block_ids.astype(np.int64) % IN_BLOCKS
    # view as [OUT_BLOCKS, 32, IN_BLOCKS, 32] and scatter per-block
    w4 = w.reshape(OUT_BLOCKS, BLOCK, IN_BLOCKS, BLOCK)
    w4[br, :, bc, :] = weight_data
    return w


def kernel(x, weight_data, bias, block_ids):
    x = np.ascontiguousarray(np.asarray(x, dtype=np.float32))
    weight_data = np.asarray(weight_data, dtype=np.float32)
    bias = np.asarray(bias, dtype=np.float32)
    block_ids = np.asarray(block_ids)

    e4 = np.dtype(ml_dtypes.float8_e4m3)
    ws_full = _dense_weight(weight_data, block_ids) * WSCALE  # [OUT, IN], scaled
    k16 = K16_TILES * P  # 3072

    # per-token-shard x in device layouts
    xts = []
    x8ts = []
    for ti in range(T_SHARDS):
        xs = x[ti * TSH : (ti + 1) * TSH, :]  # [TSH, IN]
        xT = xs.T  # [IN, TSH]
        xt = np.ascontiguousarray(
            xT[:k16].reshape(K16_TILES, P, TSH).transpose(1, 0, 2).astype(np.float16)
        )  # [P, K16_TILES, TSH]
        xts.append(xt)
        # [P, FP8_PAIRS, 2, TSH]
        x8 = np.ascontiguousarray(
            xT[k16:].reshape(FP8_PAIRS, 2, P, TSH).transpose(2, 0, 1, 3).astype(e4)
        )
        x8ts.append(x8)

    # per-outf-shard W in device layouts
    wts = []
    w8ts = []
    biases = []
    for si in range(O_SHARDS):
        ws = ws_full[si * OSH : (si + 1) * OSH, :]  # [OSH, IN], scaled
        # fp16 section: [g, ot, p, k8, o]
        wt = (
            ws[:, :k16]
            .reshape(O_TILES, P, K16_TILES // KB_SIZE, KB_SIZE, P)
            .transpose(2, 0, 4, 3, 1)
        )
        wts.append(np.ascontiguousarray(wt.astype(np.float16)))
        # fp8 section: [ot, p, pair, i, o]
        w8 = (
            ws[:, k16:]
            .reshape(O_TILES, P, FP8_PAIRS, 2, P)
            .transpose(0, 4, 2, 3, 1)
        )
        w8ts.append(np.ascontiguousarray(w8.astype(e4)))
        bs = bias[si * OSH : (si + 1) * OSH] * WSCALE  # [OSH], scaled
        biases.append(np.ascontiguousarray(bs.reshape(O_TILES, P).T))  # [P, O_TILES]

    in_maps = []
    for c in range(N_CORES):
        ti, si = c // O_SHARDS, c % O_SHARDS
        in_maps.append(
            {
                "xt": xts[ti],
                "x8": x8ts[ti],
                "wt": wts[si],
                "w8": w8ts[si],
                "bias": biases[si],
            }
        )

    nc = _build_bass()
    trace = bool(int(os.environ.get("BSL_TRACE", "0")))
    if trace:
        trace = _install_axon_ntff_hook()
    kwargs = {}
    if trace:
        tdir = os.environ.get("BSL_TRACE_DIR")
        if tdir:
            os.makedirs(tdir, exist_ok=True)
            kwargs["tmpdir"] = tdir
        kwargs["trace_cores"] = list(range(N_CORES))
    res = run_bass_kernel_spmd(
        nc,
        in_maps,
        core_ids=list(range(N_CORES)),
        trace=trace,
        **kwargs,
    )

    global LAST_EXEC_NS, LAST_RESULT
    LAST_EXEC_NS = res.exec_time_ns
    LAST_RESULT = res

    out = np.empty((N_TOKENS, OUT_FEATURES), dtype=np.float32)
    inv = np.float32(1.0 / WSCALE)
    for c in range(N_CORES):
        ti, si = c // O_SHARDS, c % O_SHARDS
        o = res.results[c]["out"]  # [O_TILES, P(o), TSH(t)]
        out[ti * TSH : (ti + 1) * TSH, si * OSH : (si + 1) * OSH] = (
            o.reshape(OSH, TSH).T * inv
        )
    return out


# revision 10
# speedup vs baseline: 1.1830x; 1.1830x over previous
"""BlockSparseLinear on 8 TRN2 NeuronCores.

Computes out = x @ W_dense.T + bias where W_dense is a [4096, 4096] matrix
assembled from 8192 nonzero 32x32 blocks (50% density).

Strategy:
  - Host: scatter the nonzero blocks into a dense weight, scale by 32 (keeps
    the fp8 section of W out of e4m3 subnormals), lay out per-core shards in
    the transposed/tiled device layout, and divide the device output by 32.
  - Sharding: 4-way over tokens x 2-way over out-features (8 cores).
    Per core: out_shard[1024 tokens, 2048 outf] = x_shard @ W_half.T + bias.
  - Mixed precision chosen from measured PE rates (fp32r 227ns, fp16 216ns,
    fp8-DoubleRow 216ns per 512-moving-row matmul, where one DoubleRow
    instruction contracts TWO 128-deep k-planes = 2x fp16 throughput):
    k-tiles 0..23 run in fp16, k-tiles 24..31 run in fp8e4m3 DoubleRow.
    Measured end-to-end rel err vs the fp32 reference: 1.879e-2 (deterministic
    for the fixed problem seed; device output matches an exact host simulation
    of this pipeline to 4e-5). Trades 128 of 1024 matmul instructions away.
  - Loop structure (from trace analysis):
      Phase A  (kb-major, fp16, k-tiles 0..15): sweep all 16 o-tiles per kb
        of 8 k-tiles, accumulate psum -> SBUF acc via DVE (bias folded in).
      Phase A2 (kb-style, fp8 DoubleRow, k-tiles 24..31): 4 DR matmuls per
        (o-tile, token-chunk), DVE-accumulated into acc.
      Phase B  (o-tile-major, fp16, k-tiles 16..23): one 8-step PSUM-resident
        accumulation per o-tile, final DVE add, then that o-tile's out DMA
        IMMEDIATELY - spreading the 8MB out flush over the whole phase
        instead of backloading it at the HBM write ceiling.
    All x is SBUF-resident before phase A2 (fp16 48KB/part + fp8 8KB/part).
  - Preamble hiding: 8 warmup matmuls on memset tiles ramp the PE p-state
    (0.65 -> 1.2 -> 2.4GHz takes ~5us of continuous busy) while the first
    real DMAs are in flight; the first two W tiles are DMA'd in per-k8 32KB
    slices so the earliest matmuls wait for 32KB each.
  - Run-to-run variance: the pod's sustained clock wanders (observed 216,
    227, 235, 259 ns per 512-row matmul across runs = 2.4 -> 2.0 GHz); all 8
    cores move together, so exec time scales accordingly.
"""

import os

import numpy as np
import ml_dtypes

import concourse.mybir as mybir
import concourse.tile as tile
from concourse import bacc
from concourse.bass_utils import run_bass_kernel_spmd

BLOCK = 32
IN_FEATURES = 4096
OUT_FEATURES = 4096
N_TOKENS = 4096
IN_BLOCKS = IN_FEATURES // BLOCK  # 128
OUT_BLOCKS = OUT_FEATURES // BLOCK  # 128

N_CORES = 8
T_SHARDS = 4  # token shards
O_SHARDS = 2  # out-feature shards
TSH = N_TOKENS // T_SHARDS  # 1024 tokens per core
OSH = OUT_FEATURES // O_SHARDS  # 2048 out features per core

P = 128  # partitions
NFREE = 512  # matmul moving free dim (one PSUM bank of fp32)
K_TILES = IN_FEATURES // P  # 32
T_CHUNKS = TSH // NFREE  # 2 moving token chunks per core
O_TILES = OSH // P  # 16 o-tiles of 128 outf
KB_SIZE = 8  # k-tiles per fp16 w tile / phase group

A_GROUPS = 2  # fp16 kb-major phases: k-tiles 0..15
B_K0 = A_GROUPS * KB_SIZE  # phase B fp16 k-tiles 16..23
FP8_K0 = B_K0 + KB_SIZE  # fp8 k-tiles 24..31
FP8_PAIRS = (K_TILES - FP8_K0) // 2  # 4 DoubleRow pairs
K16_TILES = FP8_K0  # 24 fp16 k-tiles

WSCALE = 32.0  # host-side weight scale (undone on the host after gather)
N_WARMUP_MM = 8  # p-state ramp matmuls issued before the first real one

# exec time of the slowest core from the last traced run (ns), None if untraced
LAST_EXEC_NS = None
LAST_RESULT = None


def _install_axon_ntff_hook():
    """Best-effort: register the axon NTFF profiling hook that the image's
    antenv package lacks. Returns True if tracing is possible."""
    try:
        from antenv.axon_hooks import get_axon_ntff_profile_hook

        return get_axon_ntff_profile_hook() is not None
    except ImportError:
        pass
    try:
        import sys
        import types

        import antenv
        import trn_agent_boot.trn_boot as tb

        hook = tb._ntff_profile_via_ctypes("/opt/axon/libaxon_pjrt.so")
        if hook is None:
            return False
        mod = types.ModuleType("antenv.axon_hooks")
        mod._hook = hook
        mod.get_axon_ntff_profile_hook = lambda: mod._hook
        mod.set_axon_ntff_profile_hook = lambda h: setattr(mod, "_hook", h)
        sys.modules["antenv.axon_hooks"] = mod
        antenv.axon_hooks = mod

        # avoid the artifact-upload dependency in the trace path
        import concourse.bass_utils as bu

        bu.upload_artifacts = lambda tmpdir: str(tmpdir)
        return True
    except Exception:
        return False


def _build_bass():
    nc = bacc.Bacc(None, target_bir_lowering=False)

    x_d = nc.dram_tensor(
        "xt", [P, K16_TILES, TSH], mybir.dt.float16, kind="ExternalInput"
    )
    # x8[p, pair, i, t] = x[t0 + t, (FP8_K0 + 2*pair + i)*128 + p]  (fp8)
    x8_d = nc.dram_tensor(
        "x8", [P, FP8_PAIRS, 2, TSH], mybir.dt.float8e4, kind="ExternalInput"
    )
    # wt[g, ot, p(k), k8, o] = Ws[o0 + ot*128 + o, (g*KB_SIZE + k8)*128 + p]
    w_d = nc.dram_tensor(
        "wt",
        [K16_TILES // KB_SIZE, O_TILES, P, KB_SIZE, P],
        mybir.dt.float16,
        kind="ExternalInput",
    )
    # w8[ot, p, pair, i, o] = Ws[o0 + ot*128 + o, (FP8_K0 + 2*pair + i)*128 + p]
    w8_d = nc.dram_tensor(
        "w8", [O_TILES, P, FP8_PAIRS, 2, P], mybir.dt.float8e4, kind="ExternalInput"
    )
    b_d = nc.dram_tensor("bias", [P, O_TILES], mybir.dt.float32, kind="ExternalInput")
    o_d = nc.dram_tensor(
        "out", [O_TILES, P, TSH], mybir.dt.float32, kind="ExternalOutput"
    )

    with tile.TileContext(nc) as tc:
        with (
            tc.tile_pool(name="xpool", bufs=K16_TILES * T_CHUNKS) as xpool,
            tc.tile_pool(name="x8pool", bufs=FP8_PAIRS * T_CHUNKS) as x8pool,
            tc.tile_pool(name="wpool", bufs=12) as wpool,
            tc.tile_pool(name="w8pool", bufs=4) as w8pool,
            tc.tile_pool(name="apool", bufs=1) as apool,
            tc.tile_pool(name="bpool", bufs=1) as bpool,
            tc.tile_pool(name="warm", bufs=1) as wupool,
            tc.tile_pool(name="psum", bufs=7, space="PSUM") as ppool,
            tc.tile_pool(name="psumw", bufs=1, space="PSUM") as pwpool,
        ):
            # PE p-state warmup: matmuls on memset junk, issued before any
            # real matmul; they execute while the first DMAs are in flight.
            wu_w = wupool.tile([P, P], mybir.dt.float16)
            wu_x = wupool.tile([P, NFREE], mybir.dt.float16)
            nc.gpsimd.memset(wu_w[:], 0.0)
            nc.gpsimd.memset(wu_x[:], 0.0)
            wu_ps = pwpool.tile([P, NFREE], mybir.dt.float32, tag="wu", name="wups")
            for _ in range(N_WARMUP_MM):
                nc.tensor.matmul(
                    wu_ps[:], lhsT=wu_w[:], rhs=wu_x[:], start=True, stop=True
                )

            bias_sb = bpool.tile([P, O_TILES], mybir.dt.float32)

            acc_tiles = [
                apool.tile([P, TSH], mybir.dt.float32, tag=f"a{ot}", name="acc")
                for ot in range(O_TILES)
            ]

            # fp16 x chunk tiles (k-tiles 0..23) + fp8 pair tiles, all resident
            x_tiles = [[None] * T_CHUNKS for _ in range(K16_TILES)]
            x8_tiles = [[None] * T_CHUNKS for _ in range(FP8_PAIRS)]

            def load_x(k):
                for tcn in range(T_CHUNKS):
                    x_k = xpool.tile([P, NFREE], mybir.dt.float16, tag="x", name="x")
                    nc.scalar.dma_start(
                        x_k[:], x_d[:, k, tcn * NFREE : (tcn + 1) * NFREE]
                    )
                    x_tiles[k][tcn] = x_k

            def dve_accum(ot, psums, first):
                acc = acc_tiles[ot]
                for tcn in range(T_CHUNKS):
                    sl = slice(tcn * NFREE, (tcn + 1) * NFREE)
                    if first:
                        nc.vector.tensor_tensor(
                            acc[:, sl],
                            psums[tcn][:],
                            bias_sb[:, ot : ot + 1].to_broadcast([P, NFREE]),
                            mybir.AluOpType.add,
                        )
                    else:
                        nc.vector.tensor_tensor(
                            acc[:, sl], psums[tcn][:], acc[:, sl], mybir.AluOpType.add
                        )

            # ---- Phase A: kb-major fp16 over k-tiles 0..15 ----
            for kb in range(A_GROUPS):
                for k8 in range(KB_SIZE):
                    load_x(kb * KB_SIZE + k8)
                if kb == 0:
                    nc.scalar.dma_start(bias_sb[:], b_d[:])
                if kb == A_GROUPS - 1:
                    # prefetch everything later phases need, in consumption
                    # order: fp8 pair tiles (phase A2), then phase B x
                    for pair in range(FP8_PAIRS):
                        for tcn in range(T_CHUNKS):
                            x8_k = x8pool.tile(
                                [P, 2, NFREE], mybir.dt.float8e4, tag="x8", name="x8"
                            )
                            nc.scalar.dma_start(
                                x8_k[:],
                                x8_d[:, pair, :, tcn * NFREE : (tcn + 1) * NFREE],
                            )
                            x8_tiles[pair][tcn] = x8_k
                    for k in range(B_K0, K16_TILES):
                        load_x(k)
                for ot in range(O_TILES):
                    w_sb = wpool.tile(
                        [P, KB_SIZE, P], mybir.dt.float16, tag="w", name="w"
                    )
                    if kb == 0 and ot < 2:
                        # per-k8 slices so early matmuls wait for 32KB each
                        for k8 in range(KB_SIZE):
                            nc.sync.dma_start(w_sb[:, k8], w_d[kb, ot, :, k8])
                    else:
                        nc.sync.dma_start(w_sb[:], w_d[kb, ot])
                    psums = [
                        ppool.tile([P, NFREE], mybir.dt.float32, tag="acc", name="ps")
                        for _ in range(T_CHUNKS)
                    ]
                    for k8 in range(KB_SIZE):
                        for tcn in range(T_CHUNKS):
                            nc.tensor.matmul(
                                psums[tcn][:],
                                lhsT=w_sb[:, k8],
                                rhs=x_tiles[kb * KB_SIZE + k8][tcn][:],
                                start=(k8 == 0),
                                stop=(k8 == KB_SIZE - 1),
                            )
                    dve_accum(ot, psums, first=(kb == 0))

            # ---- Phase A2: fp8 DoubleRow over k-tiles 24..31 ----
            for ot in range(O_TILES):
                w8_sb = w8pool.tile(
                    [P, FP8_PAIRS, 2, P], mybir.dt.float8e4, tag="w8", name="w8"
                )
                nc.sync.dma_start(w8_sb[:], w8_d[ot])
                psums = [
                    ppool.tile([P, NFREE], mybir.dt.float32, tag="acc", name="ps")
                    for _ in range(T_CHUNKS)
                ]
                for pair in range(FP8_PAIRS):
                    for tcn in range(T_CHUNKS):
                        nc.tensor.matmul(
                            psums[tcn][:],
                            lhsT=w8_sb[:, pair],
                            rhs=x8_tiles[pair][tcn][:],
                            start=(pair == 0),
                            stop=(pair == FP8_PAIRS - 1),
                            perf_mode=mybir.MatmulPerfMode.DoubleRow,
                        )
                dve_accum(ot, psums, first=False)

            # ---- Phase B: o-tile-major fp16 over k-tiles 16..23, out DMA per
            # o-tile as soon as it completes ----
            for ot in range(O_TILES):
                w_sb = wpool.tile([P, KB_SIZE, P], mybir.dt.float16, tag="w", name="w")
                nc.sync.dma_start(w_sb[:], w_d[A_GROUPS, ot])
                psums = [
                    ppool.tile([P, NFREE], mybir.dt.float32, tag="acc", name="ps")
                    for _ in range(T_CHUNKS)
                ]
                acc = acc_tiles[ot]
                # tcn-major: chunk 0's DVE add + out DMA overlap chunk 1's MMs
                for tcn in range(T_CHUNKS):
                    for k8 in range(KB_SIZE):
                        nc.tensor.matmul(
                            psums[tcn][:],
                            lhsT=w_sb[:, k8],
                            rhs=x_tiles[B_K0 + k8][tcn][:],
                            start=(k8 == 0),
                            stop=(k8 == KB_SIZE - 1),
                        )
                    sl = slice(tcn * NFREE, (tcn + 1) * NFREE)
                    nc.vector.tensor_tensor(
                        acc[:, sl], psums[tcn][:], acc[:, sl], mybir.AluOpType.add
                    )
                    nc.scalar.dma_start(o_d[ot, :, sl], acc[:, sl])

    nc.compile()
    return nc


def _dense_weight(weight_data, block_ids):
    """Scatter nonzero 32x32 blocks into dense [OUT, IN] (numpy, host-side)."""
    w = np.zeros((OUT_FEATURES, IN_FEATURES), dtype=np.float32)
    br = block_ids.astype(np.int64) // IN_BLOCKS
    bc = block_ids.astype(np.int64) % IN_BLOCKS
    # view as [OUT_BLOCKS, 32, IN_BLOCKS, 32] and scatter per-block
    w4 = w.reshape(OUT_BLOCKS, BLOCK, IN_BLOCKS, BLOCK)
    w4[br, :, bc, :] = weight_data
    return w


def kernel(x, weight_data, bias, block_ids):
    x = np.ascontiguousarray(np.asarray(x, dtype=np.float32))
    weight_data = np.asarray(weight_data, dtype=np.float32)
    bias = np.asarray(bias, dtype=np.float32)
    block_ids = np.asarray(block_ids)

    e4 = np.dtype(ml_dtypes.float8_e4m3)
    ws_full = _dense_weight(weight_data, block_ids) * WSCALE  # [OUT, IN], scaled
    k16 = K16_TILES * P  # 3072

    # per-token-shard x in device layouts
    xts = []
    x8ts = []
    for ti in range(T_SHARDS):
        xs = x[ti * TSH : (ti + 1) * TSH, :]  # [TSH, IN]
        xT = xs.T  # [IN, TSH]
        xt = np.ascontiguousarray(
            xT[:k16].reshape(K16_TILES, P, TSH).transpose(1, 0, 2).astype(np.float16)
        )  # [P, K16_TILES, TSH]
        xts.append(xt)
        # [P, FP8_PAIRS, 2, TSH]
        x8 = np.ascontiguousarray(
            xT[k16:].reshape(FP8_PAIRS, 2, P, TSH).transpose(2, 0, 1, 3).astype(e4)
        )
        x8ts.append(x8)

    # per-outf-shard W in device layouts
    wts = []
    w8ts = []
    biases = []
    for si in range(O_SHARDS):
        ws = ws_full[si * OSH : (si + 1) * OSH, :]  # [OSH, IN], scaled
        # fp16 section: [g, ot, p, k8, o]
        wt = (
            ws[:, :k16]
            .reshape(O_TILES, P, K16_TILES // KB_SIZE, KB_SIZE, P)
            .transpose(2, 0, 4, 3, 1)
        )
        wts.append(np.ascontiguousarray(wt.astype(np.float16)))
        # fp8 section: [ot, p, pair, i, o]
        w8 = (
            ws[:, k16:]
            .reshape(O_TILES, P, FP8_PAIRS, 2, P)
            .transpose(0, 4, 2, 3, 1)
        )
        w8ts.append(np.ascontiguousarray(w8.astype(e4)))
        bs = bias[si * OSH : (si + 1) * OSH] * WSCALE  # [OSH], scaled
        biases.append(np.ascontiguousarray(bs.reshape(O_TILES, P).T))  # [P, O_TILES]

    in_maps = []
    for c in range(N_CORES):
        ti, si = c // O_SHARDS, c % O_SHARDS
        in_maps.append(
            {
                "xt": xts[ti],
                "x8": x8ts[ti],
                "wt": wts[si],
                "w8": w8ts[si],
                "bias": biases[si],
            }
        )

    nc = _build_bass()
    trace = bool(int(os.environ.get("BSL_TRACE", "0")))
    if trace:
        trace = _install_axon_ntff_hook()
    kwargs = {}
    if trace:
        tdir = os.environ.get("BSL_TRACE_DIR")
        if tdir:
            os.makedirs(tdir, exist_ok=True)
            kwargs["tmpdir"] = tdir
        kwargs["trace_cores"] = list(range(N_CORES))
    res = run_bass_kernel_spmd(
        nc,
        in_maps,
        core_ids=list(range(N_CORES)),
        trace=trace,
        **kwargs,
    )

    global LAST_EXEC_NS, LAST_RESULT
    LAST_EXEC_NS = res.exec_time_ns
    LAST_RESULT = res

    out = np.empty((N_TOKENS, OUT_FEATURES), dtype=np.float32)
    inv = np.float32(1.0 / WSCALE)
    for c in range(N_CORES):
        ti, si = c // O_SHARDS, c % O_SHARDS
        o = res.results[c]["out"]  # [O_TILES, P(o), TSH(t)]
        out[ti * TSH : (ti + 1) * TSH, si * OSH : (si + 1) * OSH] = (
            o.reshape(OSH, TSH).T * inv
        )
    return out


# revision 11
# speedup vs baseline: 1.1847x; 1.0014x over previous
"""BlockSparseLinear on 8 TRN2 NeuronCores.

Computes out = x @ W_dense.T + bias where W_dense is a [4096, 4096] matrix
assembled from 8192 nonzero 32x32 blocks (50% density).

Strategy:
  - Host: scatter the nonzero blocks into a dense weight, scale by 32 (keeps
    the fp8 section of W out of e4m3 subnormals), lay out per-core shards in
    the transposed/tiled device layout, and divide the device output by 32.
  - Sharding: 4-way over tokens x 2-way over out-features (8 cores).
    Per core: out_shard[1024 tokens, 2048 outf] = x_shard @ W_half.T + bias.
  - Mixed precision chosen from measured PE rates (fp32r 227ns, fp16 216ns,
    fp8-DoubleRow 216ns per 512-moving-row matmul, where one DoubleRow
    instruction contracts TWO 128-deep k-planes = 2x fp16 throughput):
    k-tiles 0..23 run in fp16, k-tiles 24..31 run in fp8e4m3 DoubleRow.
    Measured end-to-end rel err vs the fp32 reference: 1.879e-2 (deterministic
    for the fixed problem seed; device output matches an exact host simulation
    of this pipeline to 4e-5). Trades 128 of 1024 matmul instructions away.
  - Loop structure (from trace analysis):
      Phase A  (kb-major, fp16, k-tiles 0..15): sweep all 16 o-tiles per kb
        of 8 k-tiles, accumulate psum -> SBUF acc via DVE (bias folded in).
      Phase A2 (kb-style, fp8 DoubleRow, k-tiles 24..31): 4 DR matmuls per
        (o-tile, token-chunk), DVE-accumulated into acc.
      Phase B  (o-tile-major, fp16, k-tiles 16..23): one 8-step PSUM-resident
        accumulation per o-tile, final DVE add, then that o-tile's out DMA
        IMMEDIATELY - spreading the 8MB out flush over the whole phase
        instead of backloading it at the HBM write ceiling.
    All x is SBUF-resident before phase A2 (fp16 48KB/part + fp8 8KB/part).
  - Preamble hiding: 8 warmup matmuls on memset tiles ramp the PE p-state
    (0.65 -> 1.2 -> 2.4GHz takes ~5us of continuous busy) while the first
    real DMAs are in flight; the first two W tiles are DMA'd in per-k8 32KB
    slices so the earliest matmuls wait for 32KB each.
  - Run-to-run variance: the pod's sustained clock wanders (observed 216,
    227, 235, 259 ns per 512-row matmul across runs = 2.4 -> 2.0 GHz); all 8
    cores move together, so exec time scales accordingly.
"""

import os

import numpy as np
import ml_dtypes

import concourse.mybir as mybir
import concourse.tile as tile
from concourse import bacc
from concourse.bass_utils import run_bass_kernel_spmd

BLOCK = 32
IN_FEATURES = 4096
OUT_FEATURES = 4096
N_TOKENS = 4096
IN_BLOCKS = IN_FEATURES // BLOCK  # 128
OUT_BLOCKS = OUT_FEATURES // BLOCK  # 128

N_CORES = 8
T_SHARDS = 4  # token shards
O_SHARDS = 2  # out-feature shards
TSH = N_TOKENS // T_SHARDS  # 1024 tokens per core
OSH = OUT_FEATURES // O_SHARDS  # 2048 out features per core

P = 128  # partitions
NFREE = 512  # matmul moving free dim (one PSUM bank of fp32)
K_TILES = IN_FEATURES // P  # 32
T_CHUNKS = TSH // NFREE  # 2 moving token chunks per core
O_TILES = OSH // P  # 16 o-tiles of 128 outf
KB_SIZE = 8  # k-tiles per fp16 w tile / phase group

A_GROUPS = 2  # fp16 kb-major phases: k-tiles 0..15
B_K0 = A_GROUPS * KB_SIZE  # phase B fp16 k-tiles 16..23
FP8_K0 = B_K0 + KB_SIZE  # fp8 k-tiles 24..31
FP8_PAIRS = (K_TILES - FP8_K0) // 2  # 4 DoubleRow pairs
K16_TILES = FP8_K0  # 24 fp16 k-tiles

WSCALE = 32.0  # host-side weight scale (undone on the host after gather)
N_WARMUP_MM = 8  # p-state ramp matmuls issued before the first real one

# exec time of the slowest core from the last traced run (ns), None if untraced
LAST_EXEC_NS = None
LAST_RESULT = None


def _install_axon_ntff_hook():
    """Best-effort: register the axon NTFF profiling hook that the image's
    antenv package lacks. Returns True if tracing is possible."""
    try:
        from antenv.axon_hooks import get_axon_ntff_profile_hook

        return get_axon_ntff_profile_hook() is not None
    except ImportError:
        pass
    try:
        import sys
        import types

        import antenv
        import trn_agent_boot.trn_boot as tb

        hook = tb._ntff_profile_via_ctypes("/opt/axon/libaxon_pjrt.so")
        if hook is None:
            return False
        mod = types.ModuleType("antenv.axon_hooks")
        mod._hook = hook
        mod.get_axon_ntff_profile_hook = lambda: mod._hook
        mod.set_axon_ntff_profile_hook = lambda h: setattr(mod, "_hook", h)
        sys.modules["antenv.axon_hooks"] = mod
        antenv.axon_hooks = mod

        # avoid the artifact-upload dependency in the trace path
        import concourse.bass_utils as bu

        bu.upload_artifacts = lambda tmpdir: str(tmpdir)
        return True
    except Exception:
        return False


def _build_bass():
    nc = bacc.Bacc(None, target_bir_lowering=False)

    x_d = nc.dram_tensor(
        "xt", [P, K16_TILES, TSH], mybir.dt.float16, kind="ExternalInput"
    )
    # x8[p, pair, i, t] = x[t0 + t, (FP8_K0 + 2*pair + i)*128 + p]  (fp8)
    x8_d = nc.dram_tensor(
        "x8", [P, FP8_PAIRS, 2, TSH], mybir.dt.float8e4, kind="ExternalInput"
    )
    # wt[g, ot, p(k), k8, o] = Ws[o0 + ot*128 + o, (g*KB_SIZE + k8)*128 + p]
    w_d = nc.dram_tensor(
        "wt",
        [K16_TILES // KB_SIZE, O_TILES, P, KB_SIZE, P],
        mybir.dt.float16,
        kind="ExternalInput",
    )
    # w8[ot, p, pair, i, o] = Ws[o0 + ot*128 + o, (FP8_K0 + 2*pair + i)*128 + p]
    w8_d = nc.dram_tensor(
        "w8", [O_TILES, P, FP8_PAIRS, 2, P], mybir.dt.float8e4, kind="ExternalInput"
    )
    b_d = nc.dram_tensor("bias", [P, O_TILES], mybir.dt.float32, kind="ExternalInput")
    o_d = nc.dram_tensor(
        "out", [O_TILES, P, TSH], mybir.dt.float32, kind="ExternalOutput"
    )

    with tile.TileContext(nc) as tc:
        with (
            tc.tile_pool(name="xpool", bufs=K16_TILES * T_CHUNKS) as xpool,
            tc.tile_pool(name="x8pool", bufs=FP8_PAIRS * T_CHUNKS) as x8pool,
            tc.tile_pool(name="wpool", bufs=12) as wpool,
            tc.tile_pool(name="w8pool", bufs=4) as w8pool,
            tc.tile_pool(name="apool", bufs=1) as apool,
            tc.tile_pool(name="bpool", bufs=1) as bpool,
            tc.tile_pool(name="warm", bufs=1) as wupool,
            tc.tile_pool(name="psum", bufs=7, space="PSUM") as ppool,
            tc.tile_pool(name="psumw", bufs=1, space="PSUM") as pwpool,
        ):
            # PE p-state warmup: matmuls on memset junk, issued before any
            # real matmul; they execute while the first DMAs are in flight.
            wu_w = wupool.tile([P, P], mybir.dt.float16)
            wu_x = wupool.tile([P, NFREE], mybir.dt.float16)
            nc.gpsimd.memset(wu_w[:], 0.0)
            nc.gpsimd.memset(wu_x[:], 0.0)
            wu_ps = pwpool.tile([P, NFREE], mybir.dt.float32, tag="wu", name="wups")
            for _ in range(N_WARMUP_MM):
                nc.tensor.matmul(
                    wu_ps[:], lhsT=wu_w[:], rhs=wu_x[:], start=True, stop=True
                )

            bias_sb = bpool.tile([P, O_TILES], mybir.dt.float32)

            acc_tiles = [
                apool.tile([P, TSH], mybir.dt.float32, tag=f"a{ot}", name="acc")
                for ot in range(O_TILES)
            ]

            # fp16 x chunk tiles (k-tiles 0..23) + fp8 pair tiles, all resident
            x_tiles = [[None] * T_CHUNKS for _ in range(K16_TILES)]
            x8_tiles = [[None] * T_CHUNKS for _ in range(FP8_PAIRS)]

            def load_x(k):
                for tcn in range(T_CHUNKS):
                    x_k = xpool.tile([P, NFREE], mybir.dt.float16, tag="x", name="x")
                    nc.scalar.dma_start(
                        x_k[:], x_d[:, k, tcn * NFREE : (tcn + 1) * NFREE]
                    )
                    x_tiles[k][tcn] = x_k

            def dve_accum(ot, psums, first):
                acc = acc_tiles[ot]
                for tcn in range(T_CHUNKS):
                    sl = slice(tcn * NFREE, (tcn + 1) * NFREE)
                    if first:
                        nc.vector.tensor_tensor(
                            acc[:, sl],
                            psums[tcn][:],
                            bias_sb[:, ot : ot + 1].to_broadcast([P, NFREE]),
                            mybir.AluOpType.add,
                        )
                    else:
                        nc.vector.tensor_tensor(
                            acc[:, sl], psums[tcn][:], acc[:, sl], mybir.AluOpType.add
                        )

            # ---- Phase A: kb-major fp16 over k-tiles 0..15 ----
            for kb in range(A_GROUPS):
                for k8 in range(KB_SIZE):
                    load_x(kb * KB_SIZE + k8)
                if kb == 0:
                    nc.scalar.dma_start(bias_sb[:], b_d[:])
                if kb == A_GROUPS - 1:
                    # prefetch everything later phases need, in consumption
                    # order: fp8 pair tiles (phase A2), then phase B x
                    for pair in range(FP8_PAIRS):
                        for tcn in range(T_CHUNKS):
                            x8_k = x8pool.tile(
                                [P, 2, NFREE], mybir.dt.float8e4, tag="x8", name="x8"
                            )
                            nc.scalar.dma_start(
                                x8_k[:],
                                x8_d[:, pair, :, tcn * NFREE : (tcn + 1) * NFREE],
                            )
                            x8_tiles[pair][tcn] = x8_k
                    for k in range(B_K0, K16_TILES):
                        load_x(k)
                for ot in range(O_TILES):
                    w_sb = wpool.tile(
                        [P, KB_SIZE, P], mybir.dt.float16, tag="w", name="w"
                    )
                    if kb == 0 and ot < 2:
                        # per-k8 slices so early matmuls wait for 32KB each
                        for k8 in range(KB_SIZE):
                            nc.sync.dma_start(w_sb[:, k8], w_d[kb, ot, :, k8])
                    else:
                        nc.sync.dma_start(w_sb[:], w_d[kb, ot])
                    psums = [
                        ppool.tile([P, NFREE], mybir.dt.float32, tag="acc", name="ps")
                        for _ in range(T_CHUNKS)
                    ]
                    for k8 in range(KB_SIZE):
                        for tcn in range(T_CHUNKS):
                            nc.tensor.matmul(
                                psums[tcn][:],
                                lhsT=w_sb[:, k8],
                                rhs=x_tiles[kb * KB_SIZE + k8][tcn][:],
                                start=(k8 == 0),
                                stop=(k8 == KB_SIZE - 1),
                            )
                    dve_accum(ot, psums, first=(kb == 0))

            # ---- Phase A2: fp8 DoubleRow over k-tiles 24..31 ----
            for ot in range(O_TILES):
                w8_sb = w8pool.tile(
                    [P, FP8_PAIRS, 2, P], mybir.dt.float8e4, tag="w8", name="w8"
                )
                nc.sync.dma_start(w8_sb[:], w8_d[ot])
                psums = [
                    ppool.tile([P, NFREE], mybir.dt.float32, tag="acc", name="ps")
                    for _ in range(T_CHUNKS)
                ]
                for pair in range(FP8_PAIRS):
                    for tcn in range(T_CHUNKS):
                        nc.tensor.matmul(
                            psums[tcn][:],
                            lhsT=w8_sb[:, pair],
                            rhs=x8_tiles[pair][tcn][:],
                            start=(pair == 0),
                            stop=(pair == FP8_PAIRS - 1),
                            perf_mode=mybir.MatmulPerfMode.DoubleRow,
                        )
                dve_accum(ot, psums, first=False)

            # ---- Phase B: o-tile-major fp16 over k-tiles 16..23, out DMA per
            # o-tile as soon as it completes ----
            for ot in range(O_TILES):
                w_sb = wpool.tile([P, KB_SIZE, P], mybir.dt.float16, tag="w", name="w")
                nc.sync.dma_start(w_sb[:], w_d[A_GROUPS, ot])
                psums = [
                    ppool.tile([P, NFREE], mybir.dt.float32, tag="acc", name="ps")
                    for _ in range(T_CHUNKS)
                ]
                acc = acc_tiles[ot]
                # tcn-major: chunk 0's DVE add + out DMA overlap chunk 1's MMs
                for tcn in range(T_CHUNKS):
                    for k8 in range(KB_SIZE):
                        nc.tensor.matmul(
                            psums[tcn][:],
                            lhsT=w_sb[:, k8],
                            rhs=x_tiles[B_K0 + k8][tcn][:],
                            start=(k8 == 0),
                            stop=(k8 == KB_SIZE - 1),
                        )
                    sl = slice(tcn * NFREE, (tcn + 1) * NFREE)
                    nc.vector.tensor_tensor(
                        acc[:, sl], psums[tcn][:], acc[:, sl], mybir.AluOpType.add
                    )
                    # split outs across both rings; halves the final-tile flush
                    eng = nc.sync if tcn == 0 else nc.scalar
                    eng.dma_start(o_d[ot, :, sl], acc[:, sl])

    nc.compile()
    return nc


def _dense_weight(weight_data, block_ids):
    """Scatter nonzero 32x32 blocks into dense [OUT, IN] (numpy, host-side)."""
    w = np.zeros((OUT_FEATURES, IN_FEATURES), dtype=np.float32)
    br = block_ids.astype(np.int64) // IN_BLOCKS
    bc = block_ids.astype(np.int64) % IN_BLOCKS
    # view as [OUT_BLOCKS, 32, IN_BLOCKS, 32] and scatter per-block
    w4 = w.reshape(OUT_BLOCKS, BLOCK, IN_BLOCKS, BLOCK)
    w4[br, :, bc, :] = weight_data
    return w


def kernel(x, weight_data, bias, block_ids):
    x = np.ascontiguousarray(np.asarray(x, dtype=np.float32))
    weight_data = np.asarray(weight_data, dtype=np.float32)
    bias = np.asarray(bias, dtype=np.float32)
    block_ids = np.asarray(block_ids)

    e4 = np.dtype(ml_dtypes.float8_e4m3)
    ws_full = _dense_weight(weight_data, block_ids) * WSCALE  # [OUT, IN], scaled
    k16 = K16_TILES * P  # 3072

    # per-token-shard x in device layouts
    xts = []
    x8ts = []
    for ti in range(T_SHARDS):
        xs = x[ti * TSH : (ti + 1) * TSH, :]  # [TSH, IN]
        xT = xs.T  # [IN, TSH]
        xt = np.ascontiguousarray(
            xT[:k16].reshape(K16_TILES, P, TSH).transpose(1, 0, 2).astype(np.float16)
        )  # [P, K16_TILES, TSH]
        xts.append(xt)
        # [P, FP8_PAIRS, 2, TSH]
        x8 = np.ascontiguousarray(
            xT[k16:].reshape(FP8_PAIRS, 2, P, TSH).transpose(2, 0, 1, 3).astype(e4)
        )
        x8ts.append(x8)

    # per-outf-shard W in device layouts
    wts = []
    w8ts = []
    biases = []
    for si in range(O_SHARDS):
        ws = ws_full[si * OSH : (si + 1) * OSH, :]  # [OSH, IN], scaled
        # fp16 section: [g, ot, p, k8, o]
        wt = (
            ws[:, :k16]
            .reshape(O_TILES, P, K16_TILES // KB_SIZE, KB_SIZE, P)
            .transpose(2, 0, 4, 3, 1)
        )
        wts.append(np.ascontiguousarray(wt.astype(np.float16)))
        # fp8 section: [ot, p, pair, i, o]
        w8 = (
            ws[:, k16:]
            .reshape(O_TILES, P, FP8_PAIRS, 2, P)
            .transpose(0, 4, 2, 3, 1)
        )
        w8ts.append(np.ascontiguousarray(w8.astype(e4)))
        bs = bias[si * OSH : (si + 1) * OSH] * WSCALE  # [OSH], scaled
        biases.append(np.ascontiguousarray(bs.reshape(O_TILES, P).T))  # [P, O_TILES]

    in_maps = []
    for c in range(N_CORES):
        ti, si = c // O_SHARDS, c % O_SHARDS
        in_maps.append(
            {
                "xt": xts[ti],
                "x8": x8ts[ti],
                "wt": wts[si],
                "w8": w8ts[si],
                "bias": biases[si],
            }
        )

    nc = _build_bass()
    trace = bool(int(os.environ.get("BSL_TRACE", "0")))
    if trace:
        trace = _install_axon_ntff_hook()
    kwargs = {}
    if trace:
        tdir = os.environ.get("BSL_TRACE_DIR")
        if tdir:
            os.makedirs(tdir, exist_ok=True)
            kwargs["tmpdir"] = tdir
        kwargs["trace_cores"] = list(range(N_CORES))
    res = run_bass_kernel_spmd(
        nc,
        in_maps,
        core_ids=list(range(N_CORES)),
        trace=trace,
        **kwargs,
    )

    global LAST_EXEC_NS, LAST_RESULT
    LAST_EXEC_NS = res.exec_time_ns
    LAST_RESULT = res

    out = np.empty((N_TOKENS, OUT_FEATURES), dtype=np.float32)
    inv = np.float32(1.0 / WSCALE)
    for c in range(N_CORES):
        ti, si = c // O_SHARDS, c % O_SHARDS
        o = res.results[c]["out"]  # [O_TILES, P(o), TSH(t)]
        out[ti * TSH : (ti + 1) * TSH, si * OSH : (si + 1) * OSH] = (
            o.reshape(OSH, TSH).T * inv
        )
    return out
